# revision 1
# baseline (speedup 1.0000x reference)
"""Trainium2 Bass kernel for the KAN-to-MLP module.

Math: out = GELU( silu(x) @ base_w.T + einsum('nhk,ohk->no', bsplines(x), spline_w * scaler) )

Reformulation: both branches fuse into ONE matmul with contraction
K = H (silu branch) + 8*H (8 B-spline basis planes) = 9216 against a
host-prepacked weight Wcat (9216, 4096).  The uniform cubic B-spline
bases are computed on-device in closed form: for u = 2.5x + 2.5,
i = floor(u), t = u - i, the only nonzero bases are planes j = i..i+3
with values  [(1-t)^3/6, (3t^3-6t^2+4)/6, (-3t^3+3t^2+3t+1)/6, t^3/6].

Sharding: data-parallel over tokens (8192 rows -> 1024/core), weights
replicated.  Per core the kernel computes features in transposed
(K x token) layout, then out(d,tok) = sum_k W(k,d)^T feat(k,tok) with
W stationary on the PE and features moving, bf16 inputs with fp32 PSUM
accumulation, GELU fused on the scalar engine.
"""

import sys

for _p in ("/opt/trn_rl_repo",):
    if _p not in sys.path:
        sys.path.insert(0, _p)

import numpy as np
import ml_dtypes

import concourse.bass as bass
import concourse.tile as tile
from concourse import bacc, mybir
from concourse.bass_utils import run_bass_kernel_spmd

AF = mybir.ActivationFunctionType
ALU = mybir.AluOpType
DT = mybir.dt

N_CORES = 8
NTOK = 1024          # tokens per core
H = 1024             # input dim
D = 4096             # output dim
NB = 8               # number of basis functions
KTOT = H + NB * H    # 9216 contraction
KT = KTOT // 128     # 72 k-tiles
CHUNK = 512          # tokens per chunk
NCHUNK = NTOK // CHUNK
DTI = D // 128       # 32 d-tiles
HT = H // 128        # 8 h-tiles

_NC_CACHE = {}


def _build_program(repeat=1):
    nc = bacc.Bacc("TRN2", target_bir_lowering=False, debug=False,
                   enable_asserts=False, num_devices=N_CORES)
    xt = nc.dram_tensor("xt", (H, NTOK), DT.float32, kind="ExternalInput").ap()
    w = nc.dram_tensor("w", (DTI, 128, KT * 128), DT.bfloat16,
                       kind="ExternalInput").ap()
    out = nc.dram_tensor("out", (D, NTOK), DT.float32, kind="ExternalOutput").ap()

    f32 = DT.float32
    bf16 = DT.bfloat16

    with tile.TileContext(nc) as tc:
        with (
            tc.tile_pool(name="xp", bufs=2) as xp,
            tc.tile_pool(name="featp", bufs=1) as featp,
            tc.tile_pool(name="scr", bufs=2) as scr,
            tc.tile_pool(name="wp", bufs=2) as wp,
            tc.tile_pool(name="psump", bufs=4, space=bass.MemorySpace.PSUM) as psump,
            tc.tile_pool(name="outp", bufs=3) as outp,
        ):
            for c in [cc for _ in range(repeat) for cc in range(NCHUNK)]:
                # ---- feature build: silu + 8 b-spline planes, (K x tok) ----
                feat = [featp.tile([128, CHUNK], bf16, tag=f"f{k}", name=f"feat{k}")
                        for k in range(KT)]
                for ht in range(HT):
                    xtile = xp.tile([128, CHUNK], f32, tag="x", name="xtile")
                    nc.sync.dma_start(
                        xtile[:], xt[ht * 128:(ht + 1) * 128,
                                     c * CHUNK:(c + 1) * CHUNK])
                    # silu branch -> feat[ht]
                    nc.scalar.activation(feat[ht][:], xtile[:], AF.Silu)
                    # u = 2.5x + 2.5 on ACT; clamp below 5 fused into DVE ops
                    u = scr.tile([128, CHUNK], f32, tag="u", name="u")
                    nc.scalar.activation(u[:], xtile[:], AF.Copy,
                                         bias=2.5, scale=2.5)
                    CL = 4.9999995
                    uc = scr.tile([128, CHUNK], f32, tag="uc", name="uc")
                    nc.vector.tensor_scalar_min(uc[:], u[:], CL)
                    # floor(u) for u in [0,5) as a sum of step functions
                    g = []
                    for v in range(1, 5):
                        gv = scr.tile([128, CHUNK], f32, tag=f"g{v}",
                                      name=f"g{v}")
                        nc.vector.tensor_scalar(gv[:], uc[:], float(v), None,
                                                ALU.is_ge)
                        g.append(gv)
                    ii = scr.tile([128, CHUNK], f32, tag="ii", name="ii")
                    nc.vector.tensor_add(ii[:], g[0][:], g[1][:])
                    nc.vector.tensor_add(ii[:], ii[:], g[2][:])
                    nc.vector.tensor_add(ii[:], ii[:], g[3][:])
                    t = scr.tile([128, CHUNK], f32, tag="t", name="t")
                    nc.vector.tensor_sub(t[:], uc[:], ii[:])
                    t2 = scr.tile([128, CHUNK], f32, tag="t2", name="t2")
                    nc.vector.tensor_mul(t2[:], t[:], t[:])
                    t3 = scr.tile([128, CHUNK], f32, tag="t3", name="t3")
                    nc.vector.tensor_mul(t3[:], t2[:], t[:])
                    s = scr.tile([128, CHUNK], f32, tag="s", name="s")
                    nc.vector.tensor_scalar(s[:], t[:], -1.0, 1.0,
                                            ALU.mult, ALU.add)
                    s2 = scr.tile([128, CHUNK], f32, tag="s2", name="s2")
                    nc.vector.tensor_mul(s2[:], s[:], s[:])
                    # b0 = (1-t)^3/6 ; b3 = t^3/6
                    b0 = scr.tile([128, CHUNK], f32, tag="b0", name="b0")
                    nc.vector.scalar_tensor_tensor(b0[:], s2[:], 1.0 / 6.0,
                                                   s[:], ALU.mult, ALU.mult)
                    b3 = scr.tile([128, CHUNK], f32, tag="b3", name="b3")
                    nc.vector.tensor_scalar_mul(b3[:], t3[:], 1.0 / 6.0)
                    # b1 = 0.5 t^3 - t^2 + 2/3
                    b1 = scr.tile([128, CHUNK], f32, tag="b1", name="b1")
                    nc.vector.scalar_tensor_tensor(b1[:], t3[:], 0.5, t2[:],
                                                   ALU.mult, ALU.subtract)
                    nc.vector.tensor_scalar_add(b1[:], b1[:], 2.0 / 3.0)
                    # b2 = 1 - b0 - b1 - b3
                    b2 = scr.tile([128, CHUNK], f32, tag="b2", name="b2")
                    nc.vector.tensor_add(b2[:], b0[:], b3[:])
                    nc.vector.tensor_add(b2[:], b2[:], b1[:])
                    nc.vector.tensor_scalar(b2[:], b2[:], -1.0, 1.0,
                                            ALU.mult, ALU.add)
                    bd = (b0, b1, b2, b3)
                    # plane j (basis index) = sum_d (i == j-d) * b_d
                    for j in range(NB):
                        dst = feat[HT + j * HT + ht]
                        terms = [(j - d, d) for d in range(4) if 0 <= j - d <= 4]
                        if len(terms) == 1:
                            iv, d = terms[0]
                            nc.vector.scalar_tensor_tensor(
                                dst[:], ii[:], float(iv), bd[d][:],
                                ALU.is_equal, ALU.mult)
                        else:
                            acc = scr.tile([128, CHUNK], f32, tag="acc",
                                           name="acc")
                            iv, d = terms[0]
                            nc.vector.scalar_tensor_tensor(
                                acc[:], ii[:], float(iv), bd[d][:],
                                ALU.is_equal, ALU.mult)
                            for n, (iv, d) in enumerate(terms[1:]):
                                last = n == len(terms) - 2
                                tgt = dst if last else acc
                                tmp = scr.tile([128, CHUNK], f32, tag="tmp",
                                               name="tmp")
                                nc.vector.scalar_tensor_tensor(
                                    tmp[:], ii[:], float(iv), bd[d][:],
                                    ALU.is_equal, ALU.mult)
                                nc.vector.tensor_add(tgt[:], acc[:], tmp[:])

                # ---- matmul sweep: W stationary, features moving ----
                for di in range(DTI):
                    wt = wp.tile([128, KT * 128], bf16, tag="w", name="wt")
                    # 4 parallel DMAs so the load spreads across queues
                    for q in range(4):
                        kspan = KT * 128 // 4
                        nc.sync.dma_start(
                            wt[:, q * kspan:(q + 1) * kspan],
                            w[di, :, q * kspan:(q + 1) * kspan])
                    ps = psump.tile([128, CHUNK], f32, tag="ps", name="ps")
                    for k in range(KT):
                        nc.tensor.matmul(ps[:], wt[:, k * 128:(k + 1) * 128],
                                         feat[k][:],
                                         start=(k == 0), stop=(k == KT - 1))
                    ot = outp.tile([128, CHUNK], f32, tag="o", name="ot")
                    nc.scalar.activation(ot[:], ps[:], AF.Gelu)
                    nc.sync.dma_start(
                        out[di * 128:(di + 1) * 128,
                            c * CHUNK:(c + 1) * CHUNK], ot[:])

    nc.compile()
    return nc


def _prep_weights(base_weight, spline_weight, spline_scaler):
    # Wcat rows: K = h (silu) then 1024 + j*1024 + h (spline plane j)
    wk = np.concatenate(
        [base_weight.T.astype(np.float32),
         (spline_weight * spline_scaler[..., None]).transpose(2, 1, 0)
         .reshape(NB * H, D)],
        axis=0)                                  # (9216, 4096)
    # -> [d_tile, kk, k*128 + dd] so each core-side W tile DMA is linear
    wt = wk.reshape(KT, 128, DTI, 128).transpose(2, 1, 0, 3) \
           .reshape(DTI, 128, KT * 128)
    return np.ascontiguousarray(wt.astype(ml_dtypes.bfloat16))


def kernel(x, base_weight, spline_weight, spline_scaler, _trace=False):
    if "nc" not in _NC_CACHE:
        _NC_CACHE["nc"] = _build_program()
    nc = _NC_CACHE["nc"]

    xf = np.asarray(x, np.float32).reshape(N_CORES * NTOK, H)
    wt = _prep_weights(np.asarray(base_weight, np.float32),
                       np.asarray(spline_weight, np.float32),
                       np.asarray(spline_scaler, np.float32))
    in_maps = []
    for c in range(N_CORES):
        xs = np.ascontiguousarray(xf[c * NTOK:(c + 1) * NTOK].T)  # (H, NTOK)
        in_maps.append({"xt": xs, "w": wt})

    res = run_bass_kernel_spmd(nc, in_maps, core_ids=list(range(N_CORES)),
                               trace=_trace)
    full = np.concatenate([res.results[c]["out"] for c in range(N_CORES)],
                          axis=1)               # (4096, 8192)
    out = np.ascontiguousarray(full.T).reshape(x.shape[0], x.shape[1], D)
    if _trace:
        kernel.last_exec_time_ns = res.exec_time_ns
        kernel.last_results = res
    return out.astype(np.float32, copy=False)


def measure_exec_ns(inputs, n=5, repeat=1):
    """Min wall time of repeated on-device executions (device-resident
    inputs) — upper bound on HW exec.  Use two `repeat` values and diff
    to cancel fixed dispatch overhead."""
    import time
    import jax
    from jax.sharding import Mesh, PartitionSpec, NamedSharding
    try:
        from jax.experimental.shard_map import shard_map
    except ImportError:
        from jax.shard_map import shard_map
    from concourse.bass2jax import (_bass_exec_p, install_neuronx_cc_hook,
                                    partition_id_tensor)

    key = f"nc{repeat}"
    if key not in _NC_CACHE:
        _NC_CACHE[key] = _build_program(repeat=repeat)
    nc = _NC_CACHE[key]
    install_neuronx_cc_hook()

    pname = (nc.partition_id_tensor.name if nc.partition_id_tensor else None)
    in_names, out_names, out_avals, zero_outs = [], [], [], []
    for alloc in nc.m.functions[0].allocations:
        if not isinstance(alloc, mybir.MemoryLocationSet):
            continue
        name = alloc.memorylocations[0].name
        if alloc.kind == "ExternalInput":
            if name != pname:
                in_names.append(name)
        elif alloc.kind == "ExternalOutput":
            out_names.append(name)
            shape = tuple(alloc.tensor_shape)
            dtype = mybir.dt.np(alloc.dtype)
            out_avals.append(jax.core.ShapedArray(shape, dtype))
            zero_outs.append(np.zeros(shape, dtype))
    n_params = len(in_names)
    all_in = in_names + out_names
    if pname is not None:
        all_in = all_in + [pname]
    donate = tuple(range(n_params, n_params + len(out_names)))

    def _body(*args):
        operands = list(args)
        if pname is not None:
            operands.append(partition_id_tensor())
        outs = _bass_exec_p.bind(
            *operands, out_avals=tuple(out_avals), in_names=tuple(all_in),
            out_names=tuple(out_names), lowering_input_output_aliases=(),
            sim_require_finite=True, sim_require_nnan=True, nc=nc)
        return tuple(outs)

    xf = np.asarray(inputs["x"], np.float32).reshape(N_CORES * NTOK, H)
    wt = _prep_weights(np.asarray(inputs["base_weight"], np.float32),
                       np.asarray(inputs["spline_weight"], np.float32),
                       np.asarray(inputs["spline_scaler"], np.float32))
    per_core = {
        "xt": [np.ascontiguousarray(xf[c * NTOK:(c + 1) * NTOK].T)
               for c in range(N_CORES)],
        "w": [wt] * N_CORES,
    }
    devices = jax.devices()[:N_CORES]
    mesh = Mesh(np.asarray(devices), ("core",))
    sh = NamedSharding(mesh, PartitionSpec("core"))
    in_specs = (PartitionSpec("core"),) * (n_params + len(out_names))
    out_specs = (PartitionSpec("core"),) * len(out_names)
    fn = jax.jit(shard_map(_body, mesh=mesh, in_specs=in_specs,
                           out_specs=out_specs, check_rep=False),
                 keep_unused=True)
    concat_in = [jax.device_put(
        np.concatenate(per_core[name], axis=0), sh) for name in in_names]
    zeros = [jax.device_put(
        np.zeros((N_CORES * z.shape[0], *z.shape[1:]), z.dtype), sh)
        for z in zero_outs]
    for a in concat_in + zeros:
        a.block_until_ready()
    times = []
    for trial in range(n):
        t0 = time.perf_counter()
        outs = fn(*concat_in, *zeros)
        for o in outs:
            o.block_until_ready()
        dt_s = time.perf_counter() - t0
        if trial > 0:        # first call includes compile
            times.append(dt_s)
    print(f"  [repeat={repeat}] per-call ms:",
          [f"{t*1e3:.2f}" for t in times])
    return int(min(times) * 1e9)



# revision 2
# speedup vs baseline: 56.3555x; 56.3555x over previous
"""Trainium2 Bass kernel for the KAN-to-MLP module.

Math: out = GELU( silu(x) @ base_w.T + einsum('nhk,ohk->no', bsplines(x), spline_w * scaler) )

Reformulation: both branches fuse into ONE matmul against a host-prepacked
weight.  The 8 cubic B-spline basis planes satisfy a partition of unity
(sum_k B_k(x) = 1 on the domain), so plane 7 is eliminated:

  spline_out[n,o] = bias[o] + sum_{h,k<7} B_k(x_nh) * (sw[o,h,k] - sw[o,h,7])
  bias[o]         = sum_h sw[o,h,7]

Contraction K = H (silu) + 7*H (spline planes) = 8192 = 64 k-tiles (vs 72
for the naive 8-plane packing).  The per-output bias is folded into the
GELU activation (ScalarE bias accepts a per-partition AP), costing zero
extra instructions.

The uniform cubic B-spline bases are computed on-device in closed form:
for u = 2.5x + 2.5, i = floor(u), t = u - i, the only nonzero bases are
planes j = i..i+3 with values [(1-t)^3/6, (3t^3-6t^2+4)/6,
(-3t^3+3t^2+3t+1)/6, t^3/6].  The basis build runs in bf16 on the DVE
(2x/4x perf modes) -- the matmul consumes bf16 anyway, and the spline is
continuous at the knots so bf16 interval assignment costs O(2^-9) abs.

Sharding: data-parallel over tokens (8192 rows -> 1024/core), weights
replicated.  Per core the kernel computes features in transposed
(K x token) layout in two 512-token chunks (chunk c+1's feature build
overlaps chunk c's matmul sweep via double-buffered feature tiles), then
out(d,tok) = sum_k W(k,d)^T feat(k,tok) with W stationary on the PE,
bf16 inputs with fp32 PSUM accumulation, GELU + bias fused on ScalarE.
"""

import sys

for _p in ("/opt/trn_rl_repo",):
    if _p not in sys.path:
        sys.path.insert(0, _p)

import numpy as np
import ml_dtypes

import concourse.bass as bass
import concourse.tile as tile
from concourse import bacc, mybir
from concourse.bass_utils import run_bass_kernel_spmd

AF = mybir.ActivationFunctionType
ALU = mybir.AluOpType
DT = mybir.dt

N_CORES = 8
NTOK = 1024          # tokens per core
H = 1024             # input dim
D = 4096             # output dim
NPL = 7              # spline planes kept (plane 7 folded into bias)
KT = H // 128 * (1 + NPL)   # 64 k-tiles, K = 8192
CHUNK = 512          # tokens per chunk
NCHUNK = NTOK // CHUNK
DTI = D // 128       # 32 d-tiles
HT = H // 128        # 8 h-tiles

_NC_CACHE = {}


def _build_program(repeat=1):
    nc = bacc.Bacc("TRN2", target_bir_lowering=False, debug=False,
                   enable_asserts=False, num_devices=N_CORES)
    xt = nc.dram_tensor("xt", (H, NTOK), DT.float32, kind="ExternalInput").ap()
    w = nc.dram_tensor("w", (DTI, 128, KT * 128), DT.bfloat16,
                       kind="ExternalInput").ap()
    bias = nc.dram_tensor("bias", (128, DTI), DT.float32,
                          kind="ExternalInput").ap()
    out = nc.dram_tensor("out", (D, NTOK), DT.float32, kind="ExternalOutput").ap()

    f32 = DT.float32
    bf16 = DT.bfloat16

    with tile.TileContext(nc) as tc:
        with (
            tc.tile_pool(name="xp", bufs=2) as xp,
            tc.tile_pool(name="featp", bufs=2) as featp,
            tc.tile_pool(name="scr", bufs=1) as scr,
            tc.tile_pool(name="wp", bufs=2) as wp,
            tc.tile_pool(name="biasp", bufs=1) as biasp,
            tc.tile_pool(name="psump", bufs=4, space=bass.MemorySpace.PSUM) as psump,
            tc.tile_pool(name="outp", bufs=3) as outp,
        ):
            bias_t = biasp.tile([128, DTI], f32, tag="bias", name="bias_t")
            nc.sync.dma_start(bias_t[:], bias[:, :])

            def body():
                for c in range(NCHUNK):
                    # ---- features: silu + 7 b-spline planes, (K x tok) bf16 ----
                    feat = [featp.tile([128, CHUNK], bf16, tag=f"f{k}",
                                       name=f"feat{k}") for k in range(KT)]
                    for ht in range(HT):
                        xtile = xp.tile([128, CHUNK], f32, tag="x", name="xtile")
                        nc.sync.dma_start(
                            xtile[:], xt[ht * 128:(ht + 1) * 128,
                                         c * CHUNK:(c + 1) * CHUNK])
                        # silu branch -> feat[ht]
                        nc.scalar.activation(feat[ht][:], xtile[:], AF.Silu)
                        # u = 2.5x + 2.5 on ACT; clamp just below 5
                        u = scr.tile([128, CHUNK], f32, tag="u", name="u")
                        nc.scalar.activation(u[:], xtile[:], AF.Copy,
                                             bias=2.5, scale=2.5)
                        uc = scr.tile([128, CHUNK], f32, tag="uc", name="uc")
                        nc.vector.tensor_scalar_min(uc[:], u[:], 4.9999995)
                        # floor(u) in [0,5) as sum of step functions (bf16 out)
                        g = []
                        for v in range(1, 5):
                            gv = scr.tile([128, CHUNK], bf16, tag=f"g{v}",
                                          name=f"g{v}")
                            nc.vector.tensor_scalar(gv[:], uc[:], float(v),
                                                    None, ALU.is_ge)
                            g.append(gv)
                        ucb = scr.tile([128, CHUNK], bf16, tag="ucb", name="ucb")
                        nc.vector.tensor_copy(ucb[:], uc[:])
                        g01 = scr.tile([128, CHUNK], bf16, tag="g01", name="g01")
                        nc.vector.tensor_add(g01[:], g[0][:], g[1][:])
                        g23 = scr.tile([128, CHUNK], bf16, tag="g23", name="g23")
                        nc.vector.tensor_add(g23[:], g[2][:], g[3][:])
                        ii = scr.tile([128, CHUNK], bf16, tag="ii", name="ii")
                        nc.vector.tensor_add(ii[:], g01[:], g23[:])
                        # t in [0,1), powers and mirrored powers (bf16)
                        t = scr.tile([128, CHUNK], bf16, tag="t", name="t")
                        nc.vector.tensor_sub(t[:], ucb[:], ii[:])
                        t2 = scr.tile([128, CHUNK], bf16, tag="t2", name="t2")
                        nc.vector.tensor_mul(t2[:], t[:], t[:])
                        t3 = scr.tile([128, CHUNK], bf16, tag="t3", name="t3")
                        nc.vector.tensor_mul(t3[:], t2[:], t[:])
                        s = scr.tile([128, CHUNK], bf16, tag="s", name="s")
                        nc.vector.tensor_scalar(s[:], t[:], -1.0, 1.0,
                                                ALU.mult, ALU.add)
                        s2 = scr.tile([128, CHUNK], bf16, tag="s2", name="s2")
                        nc.vector.tensor_mul(s2[:], s[:], s[:])
                        # b0 = (1-t)^3/6 ; b3 = t^3/6
                        b0 = scr.tile([128, CHUNK], bf16, tag="b0", name="b0")
                        nc.vector.scalar_tensor_tensor(b0[:], s2[:], 1.0 / 6.0,
                                                       s[:], ALU.mult, ALU.mult)
                        b3 = scr.tile([128, CHUNK], bf16, tag="b3", name="b3")
                        nc.vector.tensor_scalar_mul(b3[:], t3[:], 1.0 / 6.0)
                        # b1 = 0.5 t^3 - t^2 + 2/3
                        b1 = scr.tile([128, CHUNK], bf16, tag="b1", name="b1")
                        nc.vector.scalar_tensor_tensor(b1[:], t3[:], 0.5, t2[:],
                                                       ALU.mult, ALU.subtract)
                        nc.vector.tensor_scalar_add(b1[:], b1[:], 2.0 / 3.0)
                        # b2 = 1 - b0 - b1 - b3
                        b2 = scr.tile([128, CHUNK], bf16, tag="b2", name="b2")
                        nc.vector.tensor_add(b2[:], b0[:], b3[:])
                        nc.vector.tensor_add(b2[:], b2[:], b1[:])
                        nc.vector.tensor_scalar(b2[:], b2[:], -1.0, 1.0,
                                                ALU.mult, ALU.add)
                        bd = (b0, b1, b2, b3)
                        # plane j = sum_d (ii == j-d) * b_d, j = 0..6
                        for j in range(NPL):
                            dst = feat[HT + j * HT + ht]
                            terms = [(j - d, d) for d in range(4)
                                     if 0 <= j - d <= 4]
                            if len(terms) == 1:
                                iv, d = terms[0]
                                nc.vector.scalar_tensor_tensor(
                                    dst[:], ii[:], float(iv), bd[d][:],
                                    ALU.is_equal, ALU.mult)
                            else:
                                acc = scr.tile([128, CHUNK], bf16, tag="acc",
                                               name="acc")
                                iv, d = terms[0]
                                nc.vector.scalar_tensor_tensor(
                                    acc[:], ii[:], float(iv), bd[d][:],
                                    ALU.is_equal, ALU.mult)
                                for n, (iv, d) in enumerate(terms[1:]):
                                    last = n == len(terms) - 2
                                    tgt = dst if last else acc
                                    tmp = scr.tile([128, CHUNK], bf16,
                                                   tag="tmp", name="tmp")
                                    nc.vector.scalar_tensor_tensor(
                                        tmp[:], ii[:], float(iv), bd[d][:],
                                        ALU.is_equal, ALU.mult)
                                    nc.vector.tensor_add(tgt[:], acc[:], tmp[:])

                    # ---- matmul sweep: W stationary, features moving ----
                    for di in range(DTI):
                        wt = wp.tile([128, KT * 128], bf16, tag="w", name="wt")
                        for q in range(4):
                            kspan = KT * 128 // 4
                            nc.sync.dma_start(
                                wt[:, q * kspan:(q + 1) * kspan],
                                w[di, :, q * kspan:(q + 1) * kspan])
                        ps = psump.tile([128, CHUNK], f32, tag="ps", name="ps")
                        for k in range(KT):
                            nc.tensor.matmul(ps[:],
                                             wt[:, k * 128:(k + 1) * 128],
                                             feat[k][:],
                                             start=(k == 0), stop=(k == KT - 1))
                        ot = outp.tile([128, CHUNK], f32, tag="o", name="ot")
                        nc.scalar.activation(ot[:], ps[:], AF.Gelu,
                                             bias=bias_t[:, di:di + 1])
                        nc.sync.dma_start(
                            out[di * 128:(di + 1) * 128,
                                c * CHUNK:(c + 1) * CHUNK], ot[:])

            if repeat == 1:
                body()
            else:
                with tc.For_i(0, repeat, 1):
                    body()

    nc.compile()
    return nc


def _prep_weights(base_weight, spline_weight, spline_scaler):
    sw = spline_weight * spline_scaler[..., None]          # (D, H, 8)
    # rows: K = h (silu) then 1024 + j*1024 + h (spline plane j - plane 7)
    planes = (sw[:, :, :NPL] - sw[:, :, 7:8]).transpose(2, 1, 0)  # (7, H, D)
    wk = np.concatenate(
        [base_weight.T.astype(np.float32),
         planes.reshape(NPL * H, D)], axis=0)              # (8192, 4096)
    wt = wk.reshape(KT, 128, DTI, 128).transpose(2, 1, 0, 3) \
           .reshape(DTI, 128, KT * 128)
    bias = sw[:, :, 7].sum(axis=1, dtype=np.float64).astype(np.float32)  # (D,)
    bias_t = np.ascontiguousarray(bias.reshape(DTI, 128).T)  # (128, DTI)
    return np.ascontiguousarray(wt.astype(ml_dtypes.bfloat16)), bias_t


def kernel(x, base_weight, spline_weight, spline_scaler, _trace=False):
    if "nc" not in _NC_CACHE:
        _NC_CACHE["nc"] = _build_program()
    nc = _NC_CACHE["nc"]

    xf = np.asarray(x, np.float32).reshape(N_CORES * NTOK, H)
    wt, bias_t = _prep_weights(np.asarray(base_weight, np.float32),
                               np.asarray(spline_weight, np.float32),
                               np.asarray(spline_scaler, np.float32))
    in_maps = []
    for c in range(N_CORES):
        xs = np.ascontiguousarray(xf[c * NTOK:(c + 1) * NTOK].T)  # (H, NTOK)
        in_maps.append({"xt": xs, "w": wt, "bias": bias_t})

    res = run_bass_kernel_spmd(nc, in_maps, core_ids=list(range(N_CORES)),
                               trace=_trace)
    full = np.concatenate([res.results[c]["out"] for c in range(N_CORES)],
                          axis=1)               # (4096, 8192)
    out = np.ascontiguousarray(full.T).reshape(x.shape[0], x.shape[1], D)
    if _trace:
        kernel.last_exec_time_ns = res.exec_time_ns
        kernel.last_results = res
    return out.astype(np.float32, copy=False)


def measure_exec_ns(inputs, n=6, repeat=1):
    """Min wall time of repeated on-device executions (device-resident
    inputs).  A single call's wall time is dominated by dispatch/tunnel
    overhead with tens-of-ms jitter; run with two `repeat` values (the
    kernel body loops `repeat` times inside one NEFF via a hardware
    loop) and difference to isolate the true on-device body time."""
    import time
    import jax
    from jax.sharding import Mesh, PartitionSpec, NamedSharding
    try:
        from jax.experimental.shard_map import shard_map
    except ImportError:
        from jax.shard_map import shard_map
    from concourse.bass2jax import (_bass_exec_p, install_neuronx_cc_hook,
                                    partition_id_tensor)

    key = f"nc{repeat}"
    if key not in _NC_CACHE:
        _NC_CACHE[key] = _build_program(repeat=repeat)
    nc = _NC_CACHE[key]
    install_neuronx_cc_hook()

    pname = (nc.partition_id_tensor.name if nc.partition_id_tensor else None)
    in_names, out_names, out_avals, zero_outs = [], [], [], []
    for alloc in nc.m.functions[0].allocations:
        if not isinstance(alloc, mybir.MemoryLocationSet):
            continue
        name = alloc.memorylocations[0].name
        if alloc.kind == "ExternalInput":
            if name != pname:
                in_names.append(name)
        elif alloc.kind == "ExternalOutput":
            out_names.append(name)
            shape = tuple(alloc.tensor_shape)
            dtype = mybir.dt.np(alloc.dtype)
            out_avals.append(jax.core.ShapedArray(shape, dtype))
            zero_outs.append(np.zeros(shape, dtype))
    n_params = len(in_names)
    all_in = in_names + out_names
    if pname is not None:
        all_in = all_in + [pname]

    def _body(*args):
        operands = list(args)
        if pname is not None:
            operands.append(partition_id_tensor())
        outs = _bass_exec_p.bind(
            *operands, out_avals=tuple(out_avals), in_names=tuple(all_in),
            out_names=tuple(out_names), lowering_input_output_aliases=(),
            sim_require_finite=True, sim_require_nnan=True, nc=nc)
        return tuple(outs)

    xf = np.asarray(inputs["x"], np.float32).reshape(N_CORES * NTOK, H)
    wt, bias_t = _prep_weights(np.asarray(inputs["base_weight"], np.float32),
                               np.asarray(inputs["spline_weight"], np.float32),
                               np.asarray(inputs["spline_scaler"], np.float32))
    per_core = {
        "xt": [np.ascontiguousarray(xf[c * NTOK:(c + 1) * NTOK].T)
               for c in range(N_CORES)],
        "w": [wt] * N_CORES,
        "bias": [bias_t] * N_CORES,
    }
    devices = jax.devices()[:N_CORES]
    mesh = Mesh(np.asarray(devices), ("core",))
    sh = NamedSharding(mesh, PartitionSpec("core"))
    in_specs = (PartitionSpec("core"),) * (n_params + len(out_names))
    out_specs = (PartitionSpec("core"),) * len(out_names)
    fn = jax.jit(shard_map(_body, mesh=mesh, in_specs=in_specs,
                           out_specs=out_specs, check_rep=False),
                 keep_unused=True)
    concat_in = [jax.device_put(
        np.concatenate(per_core[name], axis=0), sh) for name in in_names]
    zeros = [jax.device_put(
        np.zeros((N_CORES * z.shape[0], *z.shape[1:]), z.dtype), sh)
        for z in zero_outs]
    for a in concat_in + zeros:
        a.block_until_ready()
    times = []
    for trial in range(n):
        t0 = time.perf_counter()
        outs = fn(*concat_in, *zeros)
        for o in outs:
            o.block_until_ready()
        dt_s = time.perf_counter() - t0
        if trial > 0:        # first call includes compile
            times.append(dt_s)
    print(f"  [repeat={repeat}] per-call ms:",
          [f"{t*1e3:.2f}" for t in times])
    return int(min(times) * 1e9)


# revision 20
# speedup vs baseline: 61.6971x; 1.0948x over previous
"""Trainium2 Bass kernel for the KAN-to-MLP module.

Math: out = GELU( silu(x) @ base_w.T + einsum('nhk,ohk->no', bsplines(x), spline_w * scaler) )

Reformulation: both branches fuse into ONE PSUM accumulation against
host-prepacked weights.  The 8 cubic B-spline basis planes satisfy a
partition of unity (sum_k B_k(x) = 1 on the domain), so plane 7 is
eliminated:

  spline_out[n,o] = bias[o] + sum_{h,k<7} B_k(x_nh) * (sw[o,h,k] - sw[o,h,7])
  bias[o]         = sum_h sw[o,h,7]

Contraction K = H (silu, bf16) + 7*H (spline planes, fp8) = 8192.
The per-output bias is folded into the GELU activation (ScalarE bias
accepts a per-partition AP), costing zero extra instructions.

The spline planes and their weights are quantized to fp8-e4m3 with a
global scale split (planes * SF, weights / SF, SF = 1/8 balances both
operands inside e4m3's normal range; measured end-to-end rel-err
~1.2e-2 vs the 2e-2 budget).  The 56 fp8 k-tiles run as 28 DoubleRow
matmuls (2 contraction rows per PE cell -> ~2x throughput); the 8 silu
k-tiles stay bf16.

The uniform cubic B-spline bases are computed on-device in closed form:
for u = 2.5x + 2.5, i = floor(u) (via ALU mod), t = u - i, the only
nonzero bases are planes j = i..i+3 with values [(1-t)^3/6,
(3t^3-6t^2+4)/6, (-3t^3+3t^2+3t+1)/6, t^3/6].  The basis build runs in
bf16 on the DVE (2x/4x perf modes) with squares on the otherwise-idle
ScalarE; plane tiles are built bf16 (masked scatter: one fused
is_equal*mult term, then disjoint-mask predicated overwrites) and cast
to fp8 with the SF scale folded into the cast.

Sharding: data-parallel over tokens (8192 rows -> 1024/core), weights
replicated.  Per core the kernel computes features in transposed
(K x token) layout in two 512-token chunks (chunk c+1's feature build
overlaps chunk c's matmul sweep via double-buffered feature tiles), then
out(d,tok) = sum_k W(k,d)^T feat(k,tok) with W stationary on the PE and
fp32 PSUM accumulation, GELU + bias fused on ScalarE.
"""

import sys

for _p in ("/opt/trn_rl_repo",):
    if _p not in sys.path:
        sys.path.insert(0, _p)

import numpy as np
import ml_dtypes

import concourse.bass as bass
import concourse.tile as tile
from concourse import bacc, mybir
from concourse.bass_utils import run_bass_kernel_spmd

AF = mybir.ActivationFunctionType
ALU = mybir.AluOpType
DT = mybir.dt

N_CORES = 8
NTOK = 1024          # tokens per core
H = 1024             # input dim
D = 4096             # output dim
NPL = 7              # spline planes kept (plane 7 folded into bias)
CHUNKS = (512, 512)  # per-chunk token counts (sum = NTOK)
DTI = D // 128       # 32 d-tiles
HT = H // 128        # 8 h-tiles
NPAIR = NPL * HT // 2  # 28 fp8 DoubleRow pair-tiles
SF = 0.125           # fp8 scale: planes * SF, weights / SF

# pair assignment: (j, ht) -> (pair index, element 0/1)
_PAIR = {}
for _ht in range(HT):
    for _j in range(6):
        _PAIR[(_j, _ht)] = (_ht * 3 + _j // 2, _j % 2)
for _ht in range(HT):
    _PAIR[(6, _ht)] = (24 + _ht // 2, _ht % 2)

_NC_CACHE = {}


def _build_program(repeat=1):
    nc = bacc.Bacc("TRN2", target_bir_lowering=False, debug=False,
                   enable_asserts=False, num_devices=N_CORES)
    xt = nc.dram_tensor("xt", (H, NTOK), DT.float32, kind="ExternalInput").ap()
    wbf = nc.dram_tensor("wbf", (DTI, 128, HT * 128), DT.bfloat16,
                         kind="ExternalInput").ap()
    wf8 = nc.dram_tensor("wf8", (DTI, 128, NPAIR * 256), DT.float8e4,
                         kind="ExternalInput").ap()
    bias = nc.dram_tensor("bias", (128, DTI), DT.float32,
                          kind="ExternalInput").ap()
    out = nc.dram_tensor("out", (D, NTOK), DT.float32, kind="ExternalOutput").ap()

    f32 = DT.float32
    bf16 = DT.bfloat16
    f8 = DT.float8e4
    DR = mybir.MatmulPerfMode.DoubleRow

    with tile.TileContext(nc) as tc:
        with (
            tc.tile_pool(name="xp", bufs=3) as xp,
            tc.tile_pool(name="featp", bufs=2) as featp,
            tc.tile_pool(name="scr", bufs=2) as scr,
            tc.tile_pool(name="wp", bufs=3) as wp,
            tc.tile_pool(name="biasp", bufs=1) as biasp,
            tc.tile_pool(name="psump", bufs=6, space=bass.MemorySpace.PSUM) as psump,
            tc.tile_pool(name="outp", bufs=3) as outp,
        ):
            bias_t = biasp.tile([128, DTI], f32, tag="bias", name="bias_t")
            nc.sync.dma_start(bias_t[:], bias[:, :])

            def body():
                tok0 = 0
                for c, CH in enumerate(CHUNKS):
                    # ---- features: silu (bf16) + 7 b-spline planes (fp8) ----
                    silu = [featp.tile([128, CH], bf16, tag=f"s{k}",
                                       name=f"silu{k}") for k in range(HT)]
                    pair = [featp.tile([128, 2 * CH], f8, tag=f"p{p}",
                                       name=f"pair{p}") for p in range(NPAIR)]
                    for ht in range(HT):
                        xtile = xp.tile([128, CH], f32, tag="x", name="xtile")
                        nc.sync.dma_start(
                            xtile[:], xt[ht * 128:(ht + 1) * 128,
                                         tok0:tok0 + CH])
                        # silu branch
                        nc.scalar.activation(silu[ht][:], xtile[:], AF.Silu)
                        # u = 2.5x + 2.5 on ACT; clamp just below 5
                        u = scr.tile([128, CH], f32, tag="u", name="u")
                        nc.scalar.activation(u[:], xtile[:], AF.Copy,
                                             bias=2.5, scale=2.5)
                        # clamp + floor in fp32 (bf16 u is too coarse in
                        # [4,5): 2^-6 steps would quantize t badly there);
                        # floor(u) in [0,5) as a sum of step functions
                        uc = scr.tile([128, CH], f32, tag="uc", name="uc")
                        nc.vector.tensor_scalar_min(uc[:], u[:], 4.9999995)
                        g = []
                        for v in range(1, 5):
                            gv = scr.tile([128, CH], bf16, tag=f"g{v}",
                                          name=f"g{v}")
                            nc.vector.tensor_scalar(gv[:], uc[:], float(v),
                                                    None, ALU.is_ge)
                            g.append(gv)
                        g01 = scr.tile([128, CH], bf16, tag="g01", name="g01")
                        nc.vector.tensor_add(g01[:], g[0][:], g[1][:])
                        g23 = scr.tile([128, CH], bf16, tag="g23", name="g23")
                        nc.vector.tensor_add(g23[:], g[2][:], g[3][:])
                        ii = scr.tile([128, CH], bf16, tag="ii", name="ii")
                        nc.vector.tensor_add(ii[:], g01[:], g23[:])
                        iif = scr.tile([128, CH], f32, tag="iif", name="iif")
                        nc.vector.tensor_copy(iif[:], ii[:])
                        t32 = scr.tile([128, CH], f32, tag="t32", name="t32")
                        nc.vector.tensor_sub(t32[:], uc[:], iif[:])
                        t = scr.tile([128, CH], bf16, tag="t", name="t")
                        nc.vector.tensor_copy(t[:], t32[:])
                        # interval masks e_v = (i == v)
                        masks = []
                        for v in range(5):
                            # copy_predicated requires an integer mask dtype
                            ev = scr.tile([128, CH], DT.uint8, tag=f"e{v}",
                                          name=f"e{v}")
                            nc.gpsimd.tensor_scalar(ev[:], ii[:], float(v),
                                                    None, ALU.is_equal)
                            masks.append(ev)
                        # powers on ACT (idle engine), products on DVE
                        t2 = scr.tile([128, CH], bf16, tag="t2", name="t2")
                        nc.scalar.activation(t2[:], t[:], AF.Square)
                        s = scr.tile([128, CH], bf16, tag="s", name="s")
                        nc.scalar.activation(s[:], t[:], AF.Copy,
                                             bias=1.0, scale=-1.0)
                        s2 = scr.tile([128, CH], bf16, tag="s2", name="s2")
                        nc.scalar.activation(s2[:], s[:], AF.Square)
                        t3 = scr.tile([128, CH], bf16, tag="t3", name="t3")
                        nc.vector.tensor_mul(t3[:], t2[:], t[:])
                        # b0 = (1-t)^3/6 ; b3 = t^3/6
                        b0 = scr.tile([128, CH], bf16, tag="b0", name="b0")
                        nc.vector.scalar_tensor_tensor(b0[:], s2[:], 1.0 / 6.0,
                                                       s[:], ALU.mult, ALU.mult)
                        b3 = scr.tile([128, CH], bf16, tag="b3", name="b3")
                        nc.vector.tensor_scalar_mul(b3[:], t3[:], 1.0 / 6.0)
                        # b1 = 0.5 t^3 - t^2 + 2/3
                        b1 = scr.tile([128, CH], bf16, tag="b1", name="b1")
                        nc.vector.scalar_tensor_tensor(b1[:], t3[:], 0.5, t2[:],
                                                       ALU.mult, ALU.subtract)
                        nc.vector.tensor_scalar_add(b1[:], b1[:], 2.0 / 3.0)
                        # b2 = 1 - b0 - b1 - b3
                        b2 = scr.tile([128, CH], bf16, tag="b2", name="b2")
                        nc.vector.tensor_add(b2[:], b0[:], b3[:])
                        nc.vector.tensor_add(b2[:], b2[:], b1[:])
                        nc.vector.tensor_scalar(b2[:], b2[:], -1.0, 1.0,
                                                ALU.mult, ALU.add)
                        bd = (b0, b1, b2, b3)
                        # plane j = sum_d (i == j-d) * b_d, j = 0..6, in bf16.
                        # Masks are disjoint: first term is a fused
                        # is_equal*mult, the rest masked overwrites.  Then
                        # cast *SF into the fp8 pair-tile slice.
                        for j in range(NPL):
                            pl = scr.tile([128, CH], bf16, tag="pl",
                                          name="plane")
                            terms = [(j - d, d) for d in range(4)
                                     if 0 <= j - d <= 4]
                            iv, d = terms[0]
                            nc.vector.scalar_tensor_tensor(
                                pl[:], ii[:], float(iv), bd[d][:],
                                ALU.is_equal, ALU.mult)
                            for iv, d in terms[1:]:
                                nc.vector.copy_predicated(
                                    pl[:], masks[iv][:], bd[d][:])
                            p, e = _PAIR[(j, ht)]
                            nc.vector.tensor_scalar_mul(
                                pair[p][:, e * CH:(e + 1) * CH], pl[:], SF)

                    # ---- matmul sweep: W stationary, features moving ----
                    # k-order follows production order (silu ht, then the
                    # pairs completed by that h-tile) so chunk 0's first
                    # d-tiles can start before the full feature set exists.
                    korder = []
                    for ht in range(HT):
                        korder.append(("s", ht))
                        korder += [("p", ht * 3 + q) for q in range(3)]
                        if ht % 2 == 1:
                            korder.append(("p", 24 + ht // 2))
                    nmm = len(korder)
                    for di in range(DTI):
                        wt = wp.tile([128, HT * 128], bf16, tag="wb",
                                     name="wbt")
                        nc.sync.dma_start(wt[:], wbf[di, :, :])
                        wt8 = wp.tile([128, NPAIR * 256], f8, tag="w8",
                                      name="w8t")
                        for q in range(4):
                            kspan = NPAIR * 256 // 4
                            nc.sync.dma_start(
                                wt8[:, q * kspan:(q + 1) * kspan],
                                wf8[di, :, q * kspan:(q + 1) * kspan])
                        ps = psump.tile([128, CH], f32, tag="ps", name="ps")
                        for n, (kind, idx) in enumerate(korder):
                            if kind == "s":
                                nc.tensor.matmul(
                                    ps[:], wt[:, idx * 128:(idx + 1) * 128],
                                    silu[idx][:],
                                    start=(n == 0), stop=(n == nmm - 1))
                            else:
                                lhsT = wt8[:, idx * 256:(idx + 1) * 256] \
                                    .rearrange("k (e m) -> k e m", e=2)
                                rhs = pair[idx][:].rearrange(
                                    "k (e n) -> k e n", e=2)
                                nc.tensor.matmul(
                                    ps[:], lhsT, rhs,
                                    start=(n == 0), stop=(n == nmm - 1),
                                    perf_mode=DR)
                        ot = outp.tile([128, CH], f32, tag="o", name="ot")
                        nc.scalar.activation(ot[:], ps[:], AF.Gelu,
                                             bias=bias_t[:, di:di + 1])
                        nc.sync.dma_start(
                            out[di * 128:(di + 1) * 128,
                                tok0:tok0 + CH], ot[:])
                    tok0 += CH

            if repeat == 1:
                body()
            else:
                with tc.For_i(0, repeat, 1):
                    body()

    nc.compile()
    return nc


def _prep_weights(base_weight, spline_weight, spline_scaler):
    sw = spline_weight * spline_scaler[..., None]          # (D, H, 8)
    wprime = (sw[:, :, :NPL] - sw[:, :, 7:8]) / SF         # (D, H, 7)
    # silu (bf16) part: rows k = h
    wb = base_weight.T.reshape(HT, 128, DTI, 128).transpose(2, 1, 0, 3) \
        .reshape(DTI, 128, HT * 128)
    # fp8 pairs: wsp[j, ht, hh, o]
    wsp = wprime.transpose(2, 1, 0).reshape(NPL, HT, 128, D)
    w8 = np.zeros((DTI, 128, NPAIR, 2, 128), np.float32)
    for j in range(NPL):
        for ht in range(HT):
            p, e = _PAIR[(j, ht)]
            blk = wsp[j, ht].reshape(128, DTI, 128)        # (hh, di, dd)
            w8[:, :, p, e, :] = blk.transpose(1, 0, 2)
    w8 = w8.reshape(DTI, 128, NPAIR * 256)
    bias = sw[:, :, 7].sum(axis=1, dtype=np.float64).astype(np.float32)  # (D,)
    bias_t = np.ascontiguousarray(bias.reshape(DTI, 128).T)  # (128, DTI)
    return (np.ascontiguousarray(wb.astype(ml_dtypes.bfloat16)),
            np.ascontiguousarray(w8.astype(ml_dtypes.float8_e4m3)),
            bias_t)


def kernel(x, base_weight, spline_weight, spline_scaler, _trace=False):
    if "nc" not in _NC_CACHE:
        _NC_CACHE["nc"] = _build_program()
    nc = _NC_CACHE["nc"]

    xf = np.asarray(x, np.float32).reshape(N_CORES * NTOK, H)
    wb, w8, bias_t = _prep_weights(np.asarray(base_weight, np.float32),
                                   np.asarray(spline_weight, np.float32),
                                   np.asarray(spline_scaler, np.float32))
    in_maps = []
    for c in range(N_CORES):
        xs = np.ascontiguousarray(xf[c * NTOK:(c + 1) * NTOK].T)  # (H, NTOK)
        in_maps.append({"xt": xs, "wbf": wb, "wf8": w8, "bias": bias_t})

    res = run_bass_kernel_spmd(nc, in_maps, core_ids=list(range(N_CORES)),
                               trace=_trace)
    full = np.concatenate([res.results[c]["out"] for c in range(N_CORES)],
                          axis=1)               # (4096, 8192)
    out = np.ascontiguousarray(full.T).reshape(x.shape[0], x.shape[1], D)
    if _trace:
        kernel.last_exec_time_ns = res.exec_time_ns
        kernel.last_results = res
    return out.astype(np.float32, copy=False)


def measure_exec_ns(inputs, n=6, repeat=1):
    """Min wall time of repeated on-device executions (device-resident
    inputs).  A single call's wall time is dominated by dispatch/tunnel
    overhead with tens-of-ms jitter; run with two `repeat` values (the
    kernel body loops `repeat` times inside one NEFF via a hardware
    loop) and difference to isolate the true on-device body time."""
    import time
    import jax
    from jax.sharding import Mesh, PartitionSpec, NamedSharding
    try:
        from jax.experimental.shard_map import shard_map
    except ImportError:
        from jax.shard_map import shard_map
    from concourse.bass2jax import (_bass_exec_p, install_neuronx_cc_hook,
                                    partition_id_tensor)

    key = f"nc{repeat}"
    if key not in _NC_CACHE:
        _NC_CACHE[key] = _build_program(repeat=repeat)
    nc = _NC_CACHE[key]
    install_neuronx_cc_hook()

    pname = (nc.partition_id_tensor.name if nc.partition_id_tensor else None)
    in_names, out_names, out_avals, zero_outs = [], [], [], []
    for alloc in nc.m.functions[0].allocations:
        if not isinstance(alloc, mybir.MemoryLocationSet):
            continue
        name = alloc.memorylocations[0].name
        if alloc.kind == "ExternalInput":
            if name != pname:
                in_names.append(name)
        elif alloc.kind == "ExternalOutput":
            out_names.append(name)
            shape = tuple(alloc.tensor_shape)
            dtype = mybir.dt.np(alloc.dtype)
            out_avals.append(jax.core.ShapedArray(shape, dtype))
            zero_outs.append(np.zeros(shape, dtype))
    n_params = len(in_names)
    all_in = in_names + out_names
    if pname is not None:
        all_in = all_in + [pname]

    def _body(*args):
        operands = list(args)
        if pname is not None:
            operands.append(partition_id_tensor())
        outs = _bass_exec_p.bind(
            *operands, out_avals=tuple(out_avals), in_names=tuple(all_in),
            out_names=tuple(out_names), lowering_input_output_aliases=(),
            sim_require_finite=True, sim_require_nnan=True, nc=nc)
        return tuple(outs)

    xf = np.asarray(inputs["x"], np.float32).reshape(N_CORES * NTOK, H)
    wb, w8, bias_t = _prep_weights(
        np.asarray(inputs["base_weight"], np.float32),
        np.asarray(inputs["spline_weight"], np.float32),
        np.asarray(inputs["spline_scaler"], np.float32))
    per_core = {
        "xt": [np.ascontiguousarray(xf[c * NTOK:(c + 1) * NTOK].T)
               for c in range(N_CORES)],
        "wbf": [wb] * N_CORES,
        "wf8": [w8] * N_CORES,
        "bias": [bias_t] * N_CORES,
    }
    devices = jax.devices()[:N_CORES]
    mesh = Mesh(np.asarray(devices), ("core",))
    sh = NamedSharding(mesh, PartitionSpec("core"))
    in_specs = (PartitionSpec("core"),) * (n_params + len(out_names))
    out_specs = (PartitionSpec("core"),) * len(out_names)
    fn = jax.jit(shard_map(_body, mesh=mesh, in_specs=in_specs,
                           out_specs=out_specs, check_rep=False),
                 keep_unused=True)
    concat_in = [jax.device_put(
        np.concatenate(per_core[name], axis=0), sh) for name in in_names]
    zeros = [jax.device_put(
        np.zeros((N_CORES * z.shape[0], *z.shape[1:]), z.dtype), sh)
        for z in zero_outs]
    for a in concat_in + zeros:
        a.block_until_ready()
    times = []
    for trial in range(n):
        t0 = time.perf_counter()
        outs = fn(*concat_in, *zeros)
        for o in outs:
            o.block_until_ready()
        dt_s = time.perf_counter() - t0
        if trial > 0:        # first call includes compile
            times.append(dt_s)
    print(f"  [repeat={repeat}] per-call ms:",
          [f"{t*1e3:.2f}" for t in times])
    return int(min(times) * 1e9)
